# revision 8
# baseline (speedup 1.0000x reference)
"""Trainium2 Bass kernel for nn_CMUHLayer1 (16 grouped k=2 1D convs + LeakyReLU).

Strategy: only input channels 0..53 are referenced by any group, so the whole
layer collapses to a single dense matmul with a block-sparse weight matrix:

    y[t, :] = lrelu( [x[t, 0:54], x[t+1, 0:54], 1.0] @ W_big )   W_big: [109, 1024]

W_big rows 0..53 hold the k=0 taps, rows 54..107 the k=1 taps, row 108 the
biases.  Data parallel over batch: 4 of 32 batch elements per NeuronCore.
"""

import sys

sys.path.insert(0, "/opt/trn_rl_repo")

import numpy as np

import concourse.bass as bass  # noqa: F401
import concourse.bacc as bacc
import concourse.mybir as mybir
import concourse.tile as tile
from concourse import bass_utils

GROUPS = [
    [0, 1, 2, 3, 4, 5, 26, 27, 28],
    [26, 27, 28, 29, 30, 31],
    [29, 30, 31, 32, 33, 34],
    [32, 33, 34, 35, 36, 37],
    [35, 36, 37, 38, 39, 40],
    [38, 39, 40, 41, 42, 43],
    [6, 7, 8, 9, 10],
    [9, 10, 11, 12, 13],
    [11, 12, 13, 14, 15],
    [16, 17, 18, 19, 20],
    [19, 20, 21, 22, 23],
    [21, 22, 23, 24, 25],
    [44, 45, 46, 47, 48],
    [47, 48, 49],
    [49, 50, 51, 52],
    [50, 51, 52, 53],
]

B, T, C_IN = 32, 4096, 70
NODE = 64
ALPHA = 0.2
NCORES = 8
B_LOC = B // NCORES          # 4 batch elements per core
CU = 54                      # channels actually used (0..53)
KC = 64 + CU                 # contraction rows: [tap1 0..53 | ones 54 | pad 55..63 | tap2 64..117]
NOUT = len(GROUPS) * NODE    # 1024
CHUNK = 512                  # tokens per staged chunk
JT = CHUNK // 128            # tokens per partition in the flat x load
F32 = mybir.dt.float32


def build_nc(b_loc=B_LOC, t_len=T, mm_dtype=mybir.dt.float32,
             act=mybir.ActivationFunctionType.Prelu):
    nc = bacc.Bacc("TRN2", target_bir_lowering=False, debug=False, num_devices=NCORES)
    x_d = nc.dram_tensor("x", [b_loc, t_len, C_IN], F32, kind="ExternalInput").ap()
    wb_d = nc.dram_tensor("wb", [KC, NOUT], F32, kind="ExternalInput").ap()
    id_d = nc.dram_tensor("ident", [128, 128], F32, kind="ExternalInput").ap()
    y_d = nc.dram_tensor("y", [b_loc, t_len, NOUT], F32, kind="ExternalOutput").ap()
    nch = t_len // CHUNK

    with tile.TileContext(nc) as tc:
        with (
            tc.tile_pool(name="const", bufs=1) as constp,
            tc.tile_pool(name="xin", bufs=3) as xinp,
            tc.tile_pool(name="xt", bufs=3) as xtp,
            tc.tile_pool(name="yout", bufs=3) as youtp,
            tc.tile_pool(name="pst", bufs=2, space="PSUM") as pstp,
            tc.tile_pool(name="pso", bufs=2, space="PSUM") as psop,
        ):
            wb_stage = constp.tile([KC, NOUT], F32)
            nc.sync.dma_start(wb_stage[:, :], wb_d[:, :])
            if mm_dtype == F32:
                wb_sb = wb_stage
            else:
                wb_sb = constp.tile([KC, NOUT], mm_dtype)
                nc.vector.tensor_copy(wb_sb[:, :], wb_stage[:, :])
            id_sb = constp.tile([128, 128], F32)
            nc.sync.dma_start(id_sb[:, :], id_d[:, :])

            for b in range(b_loc):
                for ci in range(nch):
                    t0 = ci * CHUNK
                    # Flat, fully-contiguous load: partition p holds tokens
                    # t0+4p .. t0+4p+3 (all 70 channels each).
                    x_sb = xinp.tile([128, JT * C_IN], F32)
                    src = x_d[b, t0 : t0 + CHUNK, :].rearrange(
                        "(p j) c -> p (j c)", p=128, j=JT
                    )
                    nc.sync.dma_start(x_sb[:, :], src)
                    # overwrite (unused) channel 54 with 1.0 -> transposes carry
                    # a ones row for the bias matmul row
                    ones_col = x_sb.rearrange("p (j c) -> p j c", j=JT)[:, :, CU : CU + 1]
                    nc.vector.memset(ones_col, 1.0)

                    # xt_sb rows: 0..53 = x[t0+u, c] (tap1), 54 = 1.0 (bias),
                    # 55..63 = junk channels (weights zero), 64..117 = x[t0+u+1, c]
                    xt_sb = xtp.tile([KC, CHUNK], mm_dtype)
                    # token u = 4p + j lives at column u; view cols as (p, j)
                    tap1 = xt_sb[0:64, :].rearrange("k (p j) -> k p j", j=JT)
                    tap2 = xt_sb[64:KC, :].rearrange("k (p j) -> k p j", j=JT)
                    for j in range(JT):
                        ps_t = pstp.tile([64, 128], F32)
                        nc.tensor.transpose(
                            ps_t[:, :],
                            x_sb[:, j * C_IN : j * C_IN + 64],
                            id_sb[:, :],
                        )
                        # tap1 columns {4p + j} (incl. ones row + junk rows)
                        nc.vector.tensor_copy(tap1[:, :, j], ps_t[:, :])
                        if j >= 1:
                            # tap2 col u needs token u+1 = 4p + j -> col 4p + (j-1)
                            nc.vector.tensor_copy(tap2[:, :, j - 1], ps_t[0:CU, :])
                        else:
                            # token 4p (p>=1) feeds tap2 col 4p-1 = 4(p-1)+3
                            nc.vector.tensor_copy(
                                tap2[:, 0:127, JT - 1], ps_t[0:CU, 1:128]
                            )
                    # halo: tap2 of the chunk's last token = first token of the
                    # next chunk (or zero-pad at the end of the sequence)
                    if ci == nch - 1:
                        nc.vector.memset(xt_sb[64:KC, CHUNK - 1 : CHUNK], 0.0)
                    else:
                        halo = x_d[b, t0 + CHUNK : t0 + CHUNK + 1, 0:CU].rearrange(
                            "t c -> c t"
                        )
                        if mm_dtype == F32:
                            nc.sync.dma_start(xt_sb[64:KC, CHUNK - 1 : CHUNK], halo)
                        else:
                            halo_sb = xinp.tile([CU, 1], F32, name="halo_sb")
                            nc.sync.dma_start(halo_sb[:, :], halo)
                            nc.vector.tensor_copy(
                                xt_sb[64:KC, CHUNK - 1 : CHUNK], halo_sb[:, :]
                            )

                    y_sb = youtp.tile([128, JT * NOUT], F32)
                    for m in range(JT):
                        ps_o = psop.tile([128, NOUT], F32)
                        lhsT = xt_sb[:, m * 128 : (m + 1) * 128]
                        for h in range(2):
                            nc.tensor.matmul(
                                ps_o[:, h * 512 : (h + 1) * 512],
                                lhsT,
                                wb_sb[:, h * 512 : (h + 1) * 512],
                                start=True,
                                stop=True,
                            )
                        nc.scalar.activation(
                            y_sb[:, m * NOUT : (m + 1) * NOUT],
                            ps_o[:, :],
                            act,
                            alpha=ALPHA,
                        )
                    dst = y_d[b, t0 : t0 + CHUNK, :].rearrange(
                        "(m q) c -> q m c", m=JT, q=128
                    )
                    nc.sync.dma_start(dst, y_sb.rearrange("q (m c) -> q m c", m=JT))

    nc.compile()
    return nc


def make_wb(Ws, bs):
    wb = np.zeros((KC, NOUT), np.float32)
    for g, idx in enumerate(GROUPS):
        for p, c in enumerate(idx):
            wb[c, g * NODE : (g + 1) * NODE] += np.asarray(Ws[g][0, p], np.float32)
            wb[64 + c, g * NODE : (g + 1) * NODE] += np.asarray(Ws[g][1, p], np.float32)
        wb[CU, g * NODE : (g + 1) * NODE] = np.asarray(bs[g], np.float32)
    return wb


_NC_CACHE = {}


def _get_nc():
    if "nc" not in _NC_CACHE:
        _NC_CACHE["nc"] = build_nc()
    return _NC_CACHE["nc"]


def run_on_hw(x, wb, trace=False):
    """x: [B, T, C_IN] float32 full input. Returns (y_full, BassKernelResults)."""
    nc = _get_nc()
    ident = np.eye(128, dtype=np.float32)
    in_maps = [
        {"x": np.ascontiguousarray(x[i * B_LOC : (i + 1) * B_LOC]), "wb": wb, "ident": ident}
        for i in range(NCORES)
    ]
    res = bass_utils.run_bass_kernel_spmd(
        nc, in_maps, core_ids=list(range(NCORES)), trace=trace
    )
    y = np.concatenate([res.results[i]["y"] for i in range(NCORES)], axis=0)
    return y, res


def kernel(**inputs):
    x = np.asarray(inputs["x"], dtype=np.float32)
    wb = make_wb([np.asarray(w, np.float32) for w in inputs["Ws"]],
                 [np.asarray(v, np.float32) for v in inputs["bs"]])
    y, _ = run_on_hw(x, wb)
    return y


# revision 10
# speedup vs baseline: 8.5352x; 8.5352x over previous
"""Trainium2 Bass kernel for nn_CMUHLayer1 (16 grouped k=2 1D convs + LeakyReLU).

Strategy: only input channels 0..53 are referenced by any group, so the whole
layer collapses to a single dense matmul with a block-sparse weight matrix:

    y[t, :] = lrelu( [x[t, 0:54], x[t+1, 0:54], 1.0] @ W_big )   W_big: [109, 1024]

W_big rows 0..53 hold the k=0 taps, rows 54..107 the k=1 taps, row 108 the
biases.  Data parallel over batch: 4 of 32 batch elements per NeuronCore.
"""

import sys

sys.path.insert(0, "/opt/trn_rl_repo")

import numpy as np

import concourse.bass as bass  # noqa: F401
import concourse.bacc as bacc
import concourse.mybir as mybir
import concourse.tile as tile
from concourse import bass_utils

GROUPS = [
    [0, 1, 2, 3, 4, 5, 26, 27, 28],
    [26, 27, 28, 29, 30, 31],
    [29, 30, 31, 32, 33, 34],
    [32, 33, 34, 35, 36, 37],
    [35, 36, 37, 38, 39, 40],
    [38, 39, 40, 41, 42, 43],
    [6, 7, 8, 9, 10],
    [9, 10, 11, 12, 13],
    [11, 12, 13, 14, 15],
    [16, 17, 18, 19, 20],
    [19, 20, 21, 22, 23],
    [21, 22, 23, 24, 25],
    [44, 45, 46, 47, 48],
    [47, 48, 49],
    [49, 50, 51, 52],
    [50, 51, 52, 53],
]

B, T, C_IN = 32, 4096, 70
NODE = 64
ALPHA = 0.2
NCORES = 8
B_LOC = B // NCORES          # 4 batch elements per core
CU = 54                      # channels actually used (0..53)
KC = 64 + CU                 # contraction rows: [tap1 0..53 | ones 54 | pad 55..63 | tap2 64..117]
NOUT = len(GROUPS) * NODE    # 1024
CHUNK = 512                  # tokens per staged chunk
JT = CHUNK // 128            # tokens per partition in the flat x load
F32 = mybir.dt.float32


def build_nc(b_loc=B_LOC, t_len=T, mm_dtype=mybir.dt.float32,
             act=mybir.ActivationFunctionType.Prelu, reps=1):
    nc = bacc.Bacc("TRN2", target_bir_lowering=False, debug=False, num_devices=NCORES)
    x_d = nc.dram_tensor("x", [b_loc, t_len, C_IN], F32, kind="ExternalInput").ap()
    wb_d = nc.dram_tensor("wb", [KC, NOUT], F32, kind="ExternalInput").ap()
    id_d = nc.dram_tensor("ident", [128, 128], F32, kind="ExternalInput").ap()
    y_d = nc.dram_tensor("y", [b_loc, t_len, NOUT], F32, kind="ExternalOutput").ap()
    nch = t_len // CHUNK

    with tile.TileContext(nc) as tc:
        with (
            tc.tile_pool(name="const", bufs=1) as constp,
            tc.tile_pool(name="xin", bufs=3) as xinp,
            tc.tile_pool(name="xt", bufs=3) as xtp,
            tc.tile_pool(name="yout", bufs=3) as youtp,
            tc.tile_pool(name="pst", bufs=2, space="PSUM") as pstp,
            tc.tile_pool(name="pso", bufs=2, space="PSUM") as psop,
        ):
            wb_stage = constp.tile([KC, NOUT], F32)
            nc.sync.dma_start(wb_stage[:, :], wb_d[:, :])
            if mm_dtype == F32:
                wb_sb = wb_stage
            else:
                wb_sb = constp.tile([KC, NOUT], mm_dtype)
                nc.vector.tensor_copy(wb_sb[:, :], wb_stage[:, :])
            id_sb = constp.tile([128, 128], F32)
            nc.sync.dma_start(id_sb[:, :], id_d[:, :])
            zero_sb = constp.tile([64, 1], F32)
            nc.vector.memset(zero_sb[:, :], 0.0)

            def body():
              for b in range(b_loc):
                for ci in range(nch):
                    t0 = ci * CHUNK
                    # Flat, fully-contiguous load: partition p holds tokens
                    # t0+4p .. t0+4p+3 (all 70 channels each).
                    x_sb = xinp.tile([128, JT * C_IN], F32)
                    src = x_d[b, t0 : t0 + CHUNK, :].rearrange(
                        "(p j) c -> p (j c)", p=128, j=JT
                    )
                    nc.sync.dma_start(x_sb[:, :], src)
                    # overwrite (unused) channel 54 with 1.0 -> transposes carry
                    # a ones row for the bias matmul row
                    ones_col = x_sb.rearrange("p (j c) -> p j c", j=JT)[:, :, CU : CU + 1]
                    nc.vector.memset(ones_col, 1.0)

                    # xt_sb rows: 0..53 = x[t0+u, c] (tap1), 54 = 1.0 (bias),
                    # 55..63 = junk channels (weights zero), 64..117 = x[t0+u+1, c]
                    xt_sb = xtp.tile([KC, CHUNK], mm_dtype)
                    # token u = 4p + j lives at column u; view cols as (p, j)
                    tap1 = xt_sb[0:64, :].rearrange("k (p j) -> k p j", j=JT)
                    tap2 = xt_sb[64:KC, :].rearrange("k (p j) -> k p j", j=JT)
                    for j in range(JT):
                        ps_t = pstp.tile([64, 128], F32)
                        nc.tensor.transpose(
                            ps_t[:, :],
                            x_sb[:, j * C_IN : j * C_IN + 64],
                            id_sb[:, :],
                        )
                        # tap1 columns {4p + j} (incl. ones row + junk rows)
                        nc.vector.tensor_copy(tap1[:, :, j], ps_t[:, :])
                        if j >= 1:
                            # tap2 col u needs token u+1 = 4p + j -> col 4p + (j-1)
                            nc.vector.tensor_copy(tap2[:, :, j - 1], ps_t[0:CU, :])
                        else:
                            # token 4p (p>=1) feeds tap2 col 4p-1 = 4(p-1)+3
                            nc.vector.tensor_copy(
                                tap2[:, 0:127, JT - 1], ps_t[0:CU, 1:128]
                            )
                    # halo: tap2 of the chunk's last token = first token of the
                    # next chunk (or zero-pad at the end of the sequence)
                    if ci == nch - 1:
                        nc.vector.tensor_copy(
                            xt_sb[64:KC, CHUNK - 1 : CHUNK], zero_sb[0:CU, :]
                        )
                    else:
                        halo = x_d[b, t0 + CHUNK : t0 + CHUNK + 1, 0:CU].rearrange(
                            "t c -> c t"
                        )
                        if mm_dtype == F32:
                            nc.sync.dma_start(xt_sb[64:KC, CHUNK - 1 : CHUNK], halo)
                        else:
                            halo_sb = xinp.tile([CU, 1], F32, name="halo_sb")
                            nc.sync.dma_start(halo_sb[:, :], halo)
                            nc.vector.tensor_copy(
                                xt_sb[64:KC, CHUNK - 1 : CHUNK], halo_sb[:, :]
                            )

                    y_sb = youtp.tile([128, JT * NOUT], F32)
                    for m in range(JT):
                        ps_o = psop.tile([128, NOUT], F32)
                        lhsT = xt_sb[:, m * 128 : (m + 1) * 128]
                        for h in range(2):
                            nc.tensor.matmul(
                                ps_o[:, h * 512 : (h + 1) * 512],
                                lhsT,
                                wb_sb[:, h * 512 : (h + 1) * 512],
                                start=True,
                                stop=True,
                            )
                        nc.scalar.activation(
                            y_sb[:, m * NOUT : (m + 1) * NOUT],
                            ps_o[:, :],
                            act,
                            alpha=ALPHA,
                        )
                    dst = y_d[b, t0 : t0 + CHUNK, :].rearrange(
                        "(m q) c -> q m c", m=JT, q=128
                    )
                    nc.sync.dma_start(dst, y_sb.rearrange("q (m c) -> q m c", m=JT))

            if reps == 1:
                body()
            else:
                with tc.For_i(0, reps, 1):
                    body()

    nc.compile()
    return nc


def make_wb(Ws, bs):
    wb = np.zeros((KC, NOUT), np.float32)
    for g, idx in enumerate(GROUPS):
        for p, c in enumerate(idx):
            wb[c, g * NODE : (g + 1) * NODE] += np.asarray(Ws[g][0, p], np.float32)
            wb[64 + c, g * NODE : (g + 1) * NODE] += np.asarray(Ws[g][1, p], np.float32)
        wb[CU, g * NODE : (g + 1) * NODE] = np.asarray(bs[g], np.float32)
    return wb


_NC_CACHE = {}


def _get_nc():
    if "nc" not in _NC_CACHE:
        _NC_CACHE["nc"] = build_nc()
    return _NC_CACHE["nc"]


def run_on_hw(x, wb, trace=False):
    """x: [B, T, C_IN] float32 full input. Returns (y_full, BassKernelResults)."""
    nc = _get_nc()
    ident = np.eye(128, dtype=np.float32)
    in_maps = [
        {"x": np.ascontiguousarray(x[i * B_LOC : (i + 1) * B_LOC]), "wb": wb, "ident": ident}
        for i in range(NCORES)
    ]
    res = bass_utils.run_bass_kernel_spmd(
        nc, in_maps, core_ids=list(range(NCORES)), trace=trace
    )
    y = np.concatenate([res.results[i]["y"] for i in range(NCORES)], axis=0)
    return y, res


def kernel(**inputs):
    x = np.asarray(inputs["x"], dtype=np.float32)
    wb = make_wb([np.asarray(w, np.float32) for w in inputs["Ws"]],
                 [np.asarray(v, np.float32) for v in inputs["bs"]])
    y, _ = run_on_hw(x, wb)
    return y


# revision 13
# speedup vs baseline: 166.3312x; 19.4877x over previous
"""Trainium2 Bass kernel for nn_CMUHLayer1 (16 grouped k=2 1D convs + LeakyReLU).

Strategy: only input channels 0..53 are referenced by any group, so the whole
layer collapses to a single dense matmul with a block-sparse weight matrix:

    y[t, :] = lrelu( [x[t, 0:54], x[t+1, 0:54], 1.0] @ W_big )   W_big: [109, 1024]

W_big rows 0..53 hold the k=0 taps, rows 54..107 the k=1 taps, row 108 the
biases.  Data parallel over batch: 4 of 32 batch elements per NeuronCore.
"""

import sys

sys.path.insert(0, "/opt/trn_rl_repo")

import numpy as np

import concourse.bass as bass  # noqa: F401
import concourse.bacc as bacc
import concourse.mybir as mybir
import concourse.tile as tile
from concourse import bass_utils

GROUPS = [
    [0, 1, 2, 3, 4, 5, 26, 27, 28],
    [26, 27, 28, 29, 30, 31],
    [29, 30, 31, 32, 33, 34],
    [32, 33, 34, 35, 36, 37],
    [35, 36, 37, 38, 39, 40],
    [38, 39, 40, 41, 42, 43],
    [6, 7, 8, 9, 10],
    [9, 10, 11, 12, 13],
    [11, 12, 13, 14, 15],
    [16, 17, 18, 19, 20],
    [19, 20, 21, 22, 23],
    [21, 22, 23, 24, 25],
    [44, 45, 46, 47, 48],
    [47, 48, 49],
    [49, 50, 51, 52],
    [50, 51, 52, 53],
]

B, T, C_IN = 32, 4096, 70
NODE = 64
ALPHA = 0.2
NCORES = 8
B_LOC = B // NCORES          # 4 batch elements per core
CU = 54                      # channels actually used (0..53)
KC = 64 + CU                 # contraction rows: [tap1 0..53 | ones 54 | pad 55..63 | tap2 64..117]
NOUT = len(GROUPS) * NODE    # 1024
CHUNK = 512                  # tokens per staged chunk
JT = CHUNK // 128            # tokens per partition in the flat x load
F32 = mybir.dt.float32


def build_nc(b_loc=B_LOC, t_len=T, mm_dtype=mybir.dt.float32r,
             act=mybir.ActivationFunctionType.Prelu, reps=1,
             y_split=4, y_bufs=32, x_bufs=10, xt_bufs=8, pso_bufs=3):
    nc = bacc.Bacc("TRN2", target_bir_lowering=False, debug=False, num_devices=NCORES)
    x_d = nc.dram_tensor("x", [b_loc, t_len, C_IN], F32, kind="ExternalInput").ap()
    wb_d = nc.dram_tensor("wb", [KC, NOUT], F32, kind="ExternalInput").ap()
    id_d = nc.dram_tensor("ident", [128, 128], F32, kind="ExternalInput").ap()
    y_d = nc.dram_tensor("y", [b_loc, t_len, NOUT], F32, kind="ExternalOutput").ap()
    nch = t_len // CHUNK

    with tile.TileContext(nc) as tc:
        with (
            tc.tile_pool(name="const", bufs=1) as constp,
            tc.tile_pool(name="xin", bufs=x_bufs) as xinp,
            tc.tile_pool(name="xt", bufs=xt_bufs) as xtp,
            tc.tile_pool(name="yout", bufs=y_bufs) as youtp,
            tc.tile_pool(name="pst", bufs=2, space="PSUM") as pstp,
            tc.tile_pool(name="pso", bufs=pso_bufs, space="PSUM") as psop,
        ):
            wb_stage = constp.tile([KC, NOUT], F32)
            nc.sync.dma_start(wb_stage[:, :], wb_d[:, :])
            if mm_dtype == F32:
                wb_sb = wb_stage
            else:
                wb_sb = constp.tile([KC, NOUT], mm_dtype)
                nc.vector.tensor_copy(wb_sb[:, :], wb_stage[:, :])
            id_sb = constp.tile([128, 128], F32)
            nc.sync.dma_start(id_sb[:, :], id_d[:, :])
            zero_sb = constp.tile([64, 1], F32)
            nc.vector.memset(zero_sb[:, :], 0.0)

            def body():
              for b in range(b_loc):
                halo_all = xinp.tile([CU, max(nch - 1, 1)], F32, name="halo_all")
                if nch > 1:
                    halo_src = x_d[b, CHUNK : t_len : CHUNK, 0:CU].rearrange(
                        "t c -> c t"
                    )
                    nc.sync.dma_start(halo_all[:, :], halo_src)
                for ci in range(nch):
                    t0 = ci * CHUNK
                    # Flat, fully-contiguous load: partition p holds tokens
                    # t0+4p .. t0+4p+3 (all 70 channels each).
                    x_sb = xinp.tile([128, JT * C_IN], F32)
                    src = x_d[b, t0 : t0 + CHUNK, :].rearrange(
                        "(p j) c -> p (j c)", p=128, j=JT
                    )
                    nc.sync.dma_start(x_sb[:, :], src)
                    # overwrite (unused) channel 54 with 1.0 -> transposes carry
                    # a ones row for the bias matmul row
                    ones_col = x_sb.rearrange("p (j c) -> p j c", j=JT)[:, :, CU : CU + 1]
                    nc.vector.memset(ones_col, 1.0)

                    # xt_sb rows: 0..53 = x[t0+u, c] (tap1), 54 = 1.0 (bias),
                    # 55..63 = junk channels (weights zero), 64..117 = x[t0+u+1, c]
                    xt_sb = xtp.tile([KC, CHUNK], mm_dtype)
                    # token u = 4p + j lives at column u; view cols as (p, j)
                    tap1 = xt_sb[0:64, :].rearrange("k (p j) -> k p j", j=JT)
                    tap2 = xt_sb[64:KC, :].rearrange("k (p j) -> k p j", j=JT)
                    for j in range(JT):
                        ps_t = pstp.tile([64, 128], F32)
                        nc.tensor.transpose(
                            ps_t[:, :],
                            x_sb[:, j * C_IN : j * C_IN + 64],
                            id_sb[:, :],
                        )
                        # tap1 columns {4p + j} (incl. ones row + junk rows)
                        nc.vector.tensor_copy(tap1[:, :, j], ps_t[:, :])
                        if j >= 1:
                            # tap2 col u needs token u+1 = 4p + j -> col 4p + (j-1)
                            nc.vector.tensor_copy(tap2[:, :, j - 1], ps_t[0:CU, :])
                        else:
                            # token 4p (p>=1) feeds tap2 col 4p-1 = 4(p-1)+3
                            nc.vector.tensor_copy(
                                tap2[:, 0:127, JT - 1], ps_t[0:CU, 1:128]
                            )
                    # halo: tap2 of the chunk's last token = first token of the
                    # next chunk (or zero-pad at the end of the sequence)
                    if ci == nch - 1:
                        nc.vector.tensor_copy(
                            xt_sb[64:KC, CHUNK - 1 : CHUNK], zero_sb[0:CU, :]
                        )
                    else:
                        nc.vector.tensor_copy(
                            xt_sb[64:KC, CHUNK - 1 : CHUNK],
                            halo_all[:, ci : ci + 1],
                        )

                    mg = JT // y_split          # m-tiles per out-DMA group
                    for g in range(y_split):
                        y_sb = youtp.tile([128, mg * NOUT], F32, name="y_sb")
                        for mi in range(mg):
                            m = g * mg + mi
                            ps_o = psop.tile([128, NOUT], F32)
                            lhsT = xt_sb[:, m * 128 : (m + 1) * 128]
                            for h in range(2):
                                nc.tensor.matmul(
                                    ps_o[:, h * 512 : (h + 1) * 512],
                                    lhsT,
                                    wb_sb[:, h * 512 : (h + 1) * 512],
                                    start=True,
                                    stop=True,
                                )
                            nc.scalar.activation(
                                y_sb[:, mi * NOUT : (mi + 1) * NOUT],
                                ps_o[:, :],
                                act,
                                alpha=ALPHA,
                            )
                        tg = t0 + g * mg * 128
                        dst = y_d[b, tg : tg + mg * 128, :].rearrange(
                            "(m q) c -> q m c", m=mg, q=128
                        )
                        nc.sync.dma_start(
                            dst, y_sb.rearrange("q (m c) -> q m c", m=mg)
                        )

            if reps == 1:
                body()
            else:
                with tc.For_i(0, reps, 1):
                    body()

    nc.compile()
    return nc


def make_wb(Ws, bs):
    wb = np.zeros((KC, NOUT), np.float32)
    for g, idx in enumerate(GROUPS):
        for p, c in enumerate(idx):
            wb[c, g * NODE : (g + 1) * NODE] += np.asarray(Ws[g][0, p], np.float32)
            wb[64 + c, g * NODE : (g + 1) * NODE] += np.asarray(Ws[g][1, p], np.float32)
        wb[CU, g * NODE : (g + 1) * NODE] = np.asarray(bs[g], np.float32)
    return wb


_NC_CACHE = {}


def _get_nc():
    if "nc" not in _NC_CACHE:
        _NC_CACHE["nc"] = build_nc()
    return _NC_CACHE["nc"]


def run_on_hw(x, wb, trace=False):
    """x: [B, T, C_IN] float32 full input. Returns (y_full, BassKernelResults)."""
    nc = _get_nc()
    ident = np.eye(128, dtype=np.float32)
    in_maps = [
        {"x": np.ascontiguousarray(x[i * B_LOC : (i + 1) * B_LOC]), "wb": wb, "ident": ident}
        for i in range(NCORES)
    ]
    res = bass_utils.run_bass_kernel_spmd(
        nc, in_maps, core_ids=list(range(NCORES)), trace=trace
    )
    y = np.concatenate([res.results[i]["y"] for i in range(NCORES)], axis=0)
    return y, res


def kernel(**inputs):
    x = np.asarray(inputs["x"], dtype=np.float32)
    wb = make_wb([np.asarray(w, np.float32) for w in inputs["Ws"]],
                 [np.asarray(v, np.float32) for v in inputs["bs"]])
    y, _ = run_on_hw(x, wb)
    return y


# revision 18
# speedup vs baseline: 202.0089x; 1.2145x over previous
"""Trainium2 Bass kernel for nn_CMUHLayer1 (16 grouped k=2 1D convs + LeakyReLU).

Strategy: only input channels 0..53 are referenced by any group, so the whole
layer collapses to a single dense matmul with a block-sparse weight matrix:

    y[t, :] = lrelu( [x[t, 0:54], x[t+1, 0:54], 1.0] @ W_big )   W_big: [109, 1024]

W_big rows 0..53 hold the k=0 taps, rows 54..107 the k=1 taps, row 108 the
biases.  Data parallel over batch: 4 of 32 batch elements per NeuronCore.
"""

import sys

sys.path.insert(0, "/opt/trn_rl_repo")

import numpy as np

import concourse.bass as bass  # noqa: F401
import concourse.bacc as bacc
import concourse.mybir as mybir
import concourse.tile as tile
from concourse import bass_utils

GROUPS = [
    [0, 1, 2, 3, 4, 5, 26, 27, 28],
    [26, 27, 28, 29, 30, 31],
    [29, 30, 31, 32, 33, 34],
    [32, 33, 34, 35, 36, 37],
    [35, 36, 37, 38, 39, 40],
    [38, 39, 40, 41, 42, 43],
    [6, 7, 8, 9, 10],
    [9, 10, 11, 12, 13],
    [11, 12, 13, 14, 15],
    [16, 17, 18, 19, 20],
    [19, 20, 21, 22, 23],
    [21, 22, 23, 24, 25],
    [44, 45, 46, 47, 48],
    [47, 48, 49],
    [49, 50, 51, 52],
    [50, 51, 52, 53],
]

B, T, C_IN = 32, 4096, 70
NODE = 64
ALPHA = 0.2
NCORES = 8
B_LOC = B // NCORES          # 4 batch elements per core
CU = 54                      # channels actually used (0..53)
KC = 64 + CU                 # contraction rows: [tap1 0..53 | ones 54 | pad 55..63 | tap2 64..117]
NOUT = len(GROUPS) * NODE    # 1024
CHUNK = 512                  # tokens per staged chunk (default)
F32 = mybir.dt.float32


def build_nc(b_loc=B_LOC, t_len=T, mm_dtype=mybir.dt.float32r,
             act=mybir.ActivationFunctionType.Prelu, reps=1,
             y_split=4, y_bufs=32, x_bufs=10, xt_bufs=8, pso_bufs=3,
             chunk=CHUNK):
    nc = bacc.Bacc("TRN2", target_bir_lowering=False, debug=False, num_devices=NCORES)
    x_d = nc.dram_tensor("x", [b_loc, t_len, C_IN], F32, kind="ExternalInput").ap()
    wb_d = nc.dram_tensor("wb", [KC, NOUT], F32, kind="ExternalInput").ap()
    id_d = nc.dram_tensor("ident", [128, 128], F32, kind="ExternalInput").ap()
    y_d = nc.dram_tensor("y", [b_loc, t_len, NOUT], F32, kind="ExternalOutput").ap()
    CHUNK = chunk
    JT = CHUNK // 128
    nch = t_len // CHUNK

    with tile.TileContext(nc) as tc:
        with (
            tc.tile_pool(name="const", bufs=1) as constp,
            tc.tile_pool(name="xin", bufs=x_bufs) as xinp,
            tc.tile_pool(name="xt", bufs=xt_bufs) as xtp,
            tc.tile_pool(name="yout", bufs=y_bufs) as youtp,
            tc.tile_pool(name="pst", bufs=2, space="PSUM") as pstp,
            tc.tile_pool(name="pso", bufs=pso_bufs, space="PSUM") as psop,
        ):
            wb_stage = constp.tile([KC, NOUT], F32)
            nc.sync.dma_start(wb_stage[:, :], wb_d[:, :])
            if mm_dtype == F32:
                wb_sb = wb_stage
            else:
                wb_sb = constp.tile([KC, NOUT], mm_dtype)
                nc.vector.tensor_copy(wb_sb[:, :], wb_stage[:, :])
            id_sb = constp.tile([128, 128], F32)
            nc.sync.dma_start(id_sb[:, :], id_d[:, :])
            zero_sb = constp.tile([64, 1], F32)
            nc.vector.memset(zero_sb[:, :], 0.0)

            def body():
              for b in range(b_loc):
                halo_all = xinp.tile([CU, max(nch - 1, 1)], F32, name="halo_all")
                if nch > 1:
                    halo_src = x_d[b, CHUNK : t_len : CHUNK, 0:CU].rearrange(
                        "t c -> c t"
                    )
                    with tc.high_priority():
                        nc.sync.dma_start(halo_all[:, :], halo_src)
                for ci in range(nch):
                    t0 = ci * CHUNK
                    # Flat, fully-contiguous load: partition p holds tokens
                    # t0+4p .. t0+4p+3 (all 70 channels each).
                    x_sb = xinp.tile([128, JT * C_IN], F32)
                    src = x_d[b, t0 : t0 + CHUNK, :].rearrange(
                        "(p j) c -> p (j c)", p=128, j=JT
                    )
                    with tc.high_priority(offset=60):
                        nc.sync.dma_start(x_sb[:, :], src)
                    # overwrite (unused) channel 54 with 1.0 -> transposes carry
                    # a ones row for the bias matmul row
                    ones_col = x_sb.rearrange("p (j c) -> p j c", j=JT)[:, :, CU : CU + 1]
                    nc.vector.memset(ones_col, 1.0)

                    # xt_sb rows: 0..53 = x[t0+u, c] (tap1), 54 = 1.0 (bias),
                    # 55..63 = junk channels (weights zero), 64..117 = x[t0+u+1, c]
                    xt_sb = xtp.tile([KC, CHUNK], mm_dtype)
                    # token u = 4p + j lives at column u; view cols as (p, j)
                    tap1 = xt_sb[0:64, :].rearrange("k (p j) -> k p j", j=JT)
                    tap2 = xt_sb[64:KC, :].rearrange("k (p j) -> k p j", j=JT)
                    for j in range(JT):
                        ps_t = pstp.tile([64, 128], F32)
                        nc.tensor.transpose(
                            ps_t[:, :],
                            x_sb[:, j * C_IN : j * C_IN + 64],
                            id_sb[:, :],
                        )
                        # tap1 columns {4p + j} (incl. ones row + junk rows)
                        nc.vector.tensor_copy(tap1[:, :, j], ps_t[:, :])
                        if j >= 1:
                            # tap2 col u needs token u+1 = 4p + j -> col 4p + (j-1)
                            nc.vector.tensor_copy(tap2[:, :, j - 1], ps_t[0:CU, :])
                        else:
                            # token 4p (p>=1) feeds tap2 col 4p-1 = 4(p-1)+3
                            nc.vector.tensor_copy(
                                tap2[:, 0:127, JT - 1], ps_t[0:CU, 1:128]
                            )
                    # halo: tap2 of the chunk's last token = first token of the
                    # next chunk (or zero-pad at the end of the sequence)
                    if ci == nch - 1:
                        nc.vector.tensor_copy(
                            xt_sb[64:KC, CHUNK - 1 : CHUNK], zero_sb[0:CU, :]
                        )
                    else:
                        nc.vector.tensor_copy(
                            xt_sb[64:KC, CHUNK - 1 : CHUNK],
                            halo_all[:, ci : ci + 1],
                        )

                    mg = JT // y_split          # m-tiles per out-DMA group
                    for g in range(y_split):
                        y_sb = youtp.tile([128, mg * NOUT], F32, name="y_sb")
                        for mi in range(mg):
                            m = g * mg + mi
                            ps_o = psop.tile([128, NOUT], F32)
                            lhsT = xt_sb[:, m * 128 : (m + 1) * 128]
                            for h in range(2):
                                nc.tensor.matmul(
                                    ps_o[:, h * 512 : (h + 1) * 512],
                                    lhsT,
                                    wb_sb[:, h * 512 : (h + 1) * 512],
                                    start=True,
                                    stop=True,
                                )
                            nc.scalar.activation(
                                y_sb[:, mi * NOUT : (mi + 1) * NOUT],
                                ps_o[:, :],
                                act,
                                alpha=ALPHA,
                            )
                        tg = t0 + g * mg * 128
                        dst = y_d[b, tg : tg + mg * 128, :].rearrange(
                            "(m q) c -> q m c", m=mg, q=128
                        )
                        nc.sync.dma_start(
                            dst, y_sb.rearrange("q (m c) -> q m c", m=mg)
                        )

            if reps == 1:
                body()
            else:
                with tc.For_i(0, reps, 1):
                    body()

    nc.compile()
    return nc


def make_wb(Ws, bs):
    wb = np.zeros((KC, NOUT), np.float32)
    for g, idx in enumerate(GROUPS):
        for p, c in enumerate(idx):
            wb[c, g * NODE : (g + 1) * NODE] += np.asarray(Ws[g][0, p], np.float32)
            wb[64 + c, g * NODE : (g + 1) * NODE] += np.asarray(Ws[g][1, p], np.float32)
        wb[CU, g * NODE : (g + 1) * NODE] = np.asarray(bs[g], np.float32)
    return wb


_NC_CACHE = {}


def _get_nc():
    if "nc" not in _NC_CACHE:
        _NC_CACHE["nc"] = build_nc()
    return _NC_CACHE["nc"]


def run_on_hw(x, wb, trace=False):
    """x: [B, T, C_IN] float32 full input. Returns (y_full, BassKernelResults)."""
    nc = _get_nc()
    ident = np.eye(128, dtype=np.float32)
    in_maps = [
        {"x": np.ascontiguousarray(x[i * B_LOC : (i + 1) * B_LOC]), "wb": wb, "ident": ident}
        for i in range(NCORES)
    ]
    res = bass_utils.run_bass_kernel_spmd(
        nc, in_maps, core_ids=list(range(NCORES)), trace=trace
    )
    y = np.concatenate([res.results[i]["y"] for i in range(NCORES)], axis=0)
    return y, res


def kernel(**inputs):
    x = np.asarray(inputs["x"], dtype=np.float32)
    wb = make_wb([np.asarray(w, np.float32) for w in inputs["Ws"]],
                 [np.asarray(v, np.float32) for v in inputs["bs"]])
    y, _ = run_on_hw(x, wb)
    return y
